# revision 23
# baseline (speedup 1.0000x reference)
"""Trainium2 Bass kernel for nn_GatedBlock (moe_routing).

Math (reference collapses): the (NB,BS,BS) reshape of weight maps block k to
rows [128k, 128k+128) of weight, so
    out[b, i] = g[b, i // 128] * (x @ W.T)[b, i] + bias[i]
with g = sigmoid(x @ gate_w + gate_b), bottom-8 of 16 gates zeroed per row.

Sharding: output-dim (i) split 8 ways -> 256 rows of W (= 2 gate blocks) per
core.  Each core receives:
  pre (128, KT, 48)   [x.T | gate_w[:, perm]] k-tile-major (gate cols permuted
                      so this core's two blocks sit at positions 0,1 -> the
                      program stays SPMD-uniform)
  rhs (128, KT, 256)  W_shard.T k-tile-major
  epi (32, 272)       [bias_shard bcast | gate_b[perm] bcast]
Gate linear runs as 16 tiny matmuls off the early `pre` load so the whole
sigmoid/top-8 chain hides under the W DMA phase; the main PSUM accumulation
then only needs two fused (psum*gate + bias) ops and the output DMA at the
tail.  Top-8 mask via vector.max + match_replace.
"""

import sys

for _p in ("/opt/trn_rl_repo", "/root/.axon_site/_ro/trn_rl_repo"):
    if _p not in sys.path:
        sys.path.append(_p)

import os as _os

import numpy as np

B = 32          # batch
D = 2048        # model dim
NB = 16         # gate blocks
BLK = D // NB   # 128 output rows per gate block
N_CORES = 8
NOUT = D // N_CORES       # 256 output cols per core
KT = D // 128             # 16 k-tiles
NPRE = B + NB             # 48 = xT cols + gate cols in the early array

MODE = _os.environ.get("GATED_MODE", "f32r")     # "f32" | "f32r" | "bf16x2"
# k-tiles per rhs DMA, summing to KT, issued round-robin over the two HWDGE
# queues.  1-tile leading groups let the PSUM accumulation start as soon as
# the first tile lands (per-queue transfers are FIFO, so a big first group
# delays every matmul); small trailing groups keep the PE chasing the tail;
# ~6 DMAs per queue stays clear of bad stalls from Tile's 4-deep
# semaphore-lane recycling.
GROUPS = [int(v) for v in
          _os.environ.get("GATED_GROUPS", "1,1,3,3,3,3,1,1").split(",")]
DMA_ENGS = _os.environ.get("GATED_ENGS", "sync,scalar").split(",")
SPLIT_OUT = _os.environ.get("GATED_SPLIT_OUT", "1") == "1"

_compiled = {}


def _build(mode):
    import concourse.bacc as bacc
    import concourse.tile as tile
    import concourse.mybir as mybir

    f32 = mybir.dt.float32
    if mode == "f32":
        mm_dt, n_split = f32, 1
    elif mode == "f32r":
        mm_dt, n_split = mybir.dt.float32r, 1
    elif mode == "bf16x2":
        mm_dt, n_split = mybir.dt.bfloat16, 2
    else:
        raise ValueError(mode)

    nc = bacc.Bacc("TRN2", target_bir_lowering=False, debug=False,
                   num_devices=N_CORES)

    prex_d = [nc.dram_tensor(f"prex{s}", [128, KT, B], mm_dt, kind="ExternalInput")
              for s in range(n_split)]
    preg_d = [nc.dram_tensor(f"preg{s}", [128, KT, NB], mm_dt, kind="ExternalInput")
              for s in range(n_split)]
    rhs_d = [nc.dram_tensor(f"rhs{s}", [128, KT, NOUT], mm_dt, kind="ExternalInput")
             for s in range(n_split)]
    epi_d = nc.dram_tensor("epi", [B, NOUT + NB], f32, kind="ExternalInput")
    out_d = nc.dram_tensor("out", [B, NOUT], f32, kind="ExternalOutput")

    with tile.TileContext(nc) as tc:
        with (
            tc.tile_pool(name="sb", bufs=1) as sb,
            tc.tile_pool(name="ps", bufs=1, space="PSUM") as psp,
        ):
            prex = [sb.tile([128, KT, B], mm_dt, name=f"prex_sb{s}", tag=f"prex_sb{s}")
                    for s in range(n_split)]
            preg = [sb.tile([128, KT, NB], mm_dt, name=f"preg_sb{s}", tag=f"preg_sb{s}")
                    for s in range(n_split)]
            rhs = [sb.tile([128, KT, NOUT], mm_dt, name=f"rhs_sb{s}", tag=f"rhs_sb{s}")
                   for s in range(n_split)]
            epi = sb.tile([B, NOUT + NB], f32, name="epi_sb", tag="epi_sb")
            graw = sb.tile([B, NB], f32, name="graw", tag="graw")
            g = sb.tile([B, NB], f32, name="g", tag="g")
            m8 = sb.tile([B, 8], f32, name="m8", tag="m8")
            rep = sb.tile([B, NB], f32, name="rep", tag="rep")
            gk = sb.tile([B, NB], f32, name="gk", tag="gk")
            outt = sb.tile([B, NOUT], f32, name="outt", tag="outt")
            ps_g = psp.tile([B, NB], f32, name="ps_g", tag="ps_g")
            ps_m = [psp.tile([B, BLK], f32, name=f"ps_m{h}", tag=f"ps_m{h}")
                    for h in range(NOUT // BLK)]

            engs = [getattr(nc, e) for e in DMA_ENGS]

            # early loads split across queues: x.T (matmul stationary, needed
            # first) leads queue 0; gate_w + epi lead queue 1
            for s in range(n_split):
                engs[0].dma_start(prex[s][:], prex_d[s].ap())
                engs[-1].dma_start(preg[s][:], preg_d[s].ap())
            engs[-1].dma_start(epi[:], epi_d.ap())

            # rhs groups, round-robin over DMA queues
            assert sum(GROUPS) == KT, GROUPS
            di = 0
            for s in range(n_split):
                t0 = 0
                for gsz in GROUPS:
                    engs[di % len(engs)].dma_start(
                        rhs[s][:, t0:t0 + gsz, :],
                        rhs_d[s].ap()[:, t0:t0 + gsz, :],
                    )
                    t0 += gsz
                    di += 1

            # accumulation passes: f32/f32r -> [(0,0)]; bf16x2 -> hh, hl, lh
            passes = [(0, 0)] if n_split == 1 else [(0, 0), (0, 1), (1, 0)]
            n_mm = len(passes) * KT

            # gate linear: 16 tiny matmuls off the early load only
            i = 0
            for (sx, sw) in passes:
                for t in range(KT):
                    nc.tensor.matmul(
                        ps_g[:], prex[sx][:, t, :], preg[sw][:, t, :],
                        start=(i == 0), stop=(i == n_mm - 1),
                    )
                    i += 1

            # gate chain (hides under the rhs DMA phase)
            nc.vector.tensor_add(graw[:], ps_g[:], epi[:, NOUT:NOUT + NB])
            nc.scalar.activation(g[:], graw[:],
                                 mybir.ActivationFunctionType.Sigmoid)
            nc.vector.max(m8[:], g[:])
            nc.vector.match_replace(rep[:], m8[:], g[:], 0.0)
            nc.vector.tensor_sub(gk[:], g[:], rep[:])

            # main matmul accumulation: two independent column-half chains so
            # the first half's epilogue + store can start one matmul earlier.
            # Per k-tile, half B runs before half A so chain A's last matmul
            # is the overall second-to-last.
            nh = NOUT // BLK
            i = [0] * nh
            for (sx, sw) in passes:
                for t in range(KT):
                    for h in reversed(range(nh)):
                        nc.tensor.matmul(
                            ps_m[h][:], prex[sx][:, t, :],
                            rhs[sw][:, t, h * BLK:(h + 1) * BLK],
                            start=(i[h] == 0), stop=(i[h] == n_mm - 1),
                        )
                        i[h] += 1

            # out = psum * g[block] + bias; each half's store issues as soon
            # as that half is ready
            for h in range(nh):
                sl = slice(h * BLK, (h + 1) * BLK)
                nc.vector.scalar_tensor_tensor(
                    outt[:, sl], ps_m[h][:], gk[:, h:h + 1], epi[:, sl],
                    mybir.AluOpType.mult, mybir.AluOpType.add,
                )
                if SPLIT_OUT:
                    engs[h % len(engs)].dma_start(out_d.ap()[:, sl], outt[:, sl])
            if not SPLIT_OUT:
                nc.sync.dma_start(out_d.ap(), outt[:])

    nc.compile()
    return nc


def _tile_major(a):
    """(D, n) -> (128, KT, n) k-tile-major contiguous."""
    n = a.shape[1]
    return np.ascontiguousarray(a.reshape(KT, 128, n).transpose(1, 0, 2))


def _split_parts(a, mode):
    """Split fp32 array into matmul-dtype parts per MODE."""
    if mode == "f32" or mode == "f32r":
        return [np.ascontiguousarray(a, dtype=np.float32)]
    import ml_dtypes
    hi = a.astype(ml_dtypes.bfloat16)
    lo = (a - hi.astype(np.float32)).astype(ml_dtypes.bfloat16)
    return [hi, lo]


def build_in_maps(x, gate_w, gate_b, weight, bias):
    x = np.asarray(x, dtype=np.float32)
    gate_w = np.asarray(gate_w, dtype=np.float32)
    gate_b = np.asarray(gate_b, dtype=np.float32)
    weight = np.asarray(weight, dtype=np.float32)
    bias = np.asarray(bias, dtype=np.float32)

    in_maps = []
    for c in range(N_CORES):
        perm = [2 * c, 2 * c + 1] + [k for k in range(NB)
                                     if k not in (2 * c, 2 * c + 1)]
        prex_parts = [_tile_major(p)
                      for p in _split_parts(np.ascontiguousarray(x.T), MODE)]
        preg_parts = [_tile_major(p)
                      for p in _split_parts(
                          np.ascontiguousarray(gate_w[:, perm]), MODE)]
        w_shard = weight[c * NOUT:(c + 1) * NOUT, :]              # (256, 2048)
        rhs_parts = [_tile_major(p)
                     for p in _split_parts(np.ascontiguousarray(w_shard.T), MODE)]
        epi = np.concatenate([
            np.broadcast_to(bias[c * NOUT:(c + 1) * NOUT], (B, NOUT)),
            np.broadcast_to(gate_b[perm], (B, NB)),
        ], axis=1).astype(np.float32)
        m = {"epi": np.ascontiguousarray(epi)}
        for s, (xp, gp, rp) in enumerate(zip(prex_parts, preg_parts, rhs_parts)):
            m[f"prex{s}"] = xp
            m[f"preg{s}"] = gp
            m[f"rhs{s}"] = rp
        in_maps.append(m)
    return in_maps


def _ensure_ntff_hook():
    """If a caller sets BASS_TRACE, run_bass_kernel_spmd imports
    antenv.axon_hooks, which is missing in this image; provide a working
    ctypes-backed stub so tracing degrades gracefully instead of raising."""
    try:
        from antenv.axon_hooks import get_axon_ntff_profile_hook  # noqa: F401
        return
    except ImportError:
        pass
    import contextlib
    import ctypes
    import types

    try:
        lib = ctypes.CDLL("/opt/axon/libaxon_pjrt.so")
        assert hasattr(lib, "axon_start_nrt_profile")
        lib.axon_start_nrt_profile.argtypes = [
            ctypes.POINTER(ctypes.c_int64), ctypes.c_size_t]
        lib.axon_start_nrt_profile.restype = ctypes.c_int64
        lib.axon_stop_nrt_profile.argtypes = [ctypes.c_char_p]
        lib.axon_stop_nrt_profile.restype = ctypes.c_int64

        @contextlib.contextmanager
        def _hook(output_dir, device_ids):
            import jax
            jax.devices()
            if device_ids:
                ids = (ctypes.c_int64 * len(device_ids))(*device_ids)
                rc = lib.axon_start_nrt_profile(ids, len(device_ids))
            else:
                rc = lib.axon_start_nrt_profile(None, 0)
            if rc != 0:
                raise RuntimeError(f"axon_start_nrt_profile rc={rc}")
            try:
                yield
            finally:
                lib.axon_stop_nrt_profile(str(output_dir).encode())

        hook = _hook
    except Exception:
        hook = None

    mod = types.ModuleType("antenv.axon_hooks")
    mod.get_axon_ntff_profile_hook = lambda: hook
    mod.set_axon_ntff_profile_hook = lambda h: None
    sys.modules["antenv.axon_hooks"] = mod


def kernel(x, gate_w, gate_b, weight, bias):
    _ensure_ntff_hook()
    from concourse.bass_utils import run_bass_kernel_spmd

    if MODE not in _compiled:
        _compiled[MODE] = _build(MODE)
    nc = _compiled[MODE]

    in_maps = build_in_maps(x, gate_w, gate_b, weight, bias)
    res = run_bass_kernel_spmd(nc, in_maps, list(range(N_CORES)))
    out = np.concatenate([res.results[c]["out"] for c in range(N_CORES)], axis=1)
    return out.astype(np.float32)
